# revision 1
# baseline (speedup 1.0000x reference)
"""MultiHeadAttnBlock TRN2 kernel.

Full inputs -> shard across 8 NeuronCores -> full output.

Sharding: core i handles (batch b = i//4, spatial quarter sq = i%4).
Each core computes group-norm stats for its batch, normalizes x/y with the
per-channel affine (A, B) derived from the group stats, computes K/V over
the full spatial dim and Q over its quarter, runs 4-head attention for its
1024 query positions against all 4096 keys, projects with wo, and adds the
residual.  The host slices inputs and concatenates the 8 [256, 1024]
outputs.

Layout:
 - q, k in [c, s] "conv layout" straight out of the 1x1-conv matmul.
 - scores computed transposed: scT[t, s] = k[d, t-tile].T @ q[d, s];
   the two heads of a pair live at partitions 0-63 / 64-127 and share the
   PE array via row tiling.
 - exp on ScalarE - the kernel bottleneck (16.8M exps/core).
 - attn@v: out.T[d', s] = v'[t, d'].T @ expT[t, s] accumulated over the 32
   t-tiles in PSUM, where v' = [v | ones]: column 64 accumulates the
   softmax denominator for free.
 - the denominator reciprocal uses the DVE 32x32-transpose trick to spread
   4096 values across 32 lanes; the broadcast back to 64 partitions is a
   K=1 matmul written into the (already drained) accumulator PSUM so no
   extra PSUM bank is needed and the next pair's tiles are never blocked.
"""

import numpy as np
import ml_dtypes

import concourse.bass as bass
import concourse.mybir as mybir
import bass_rust as _br
from concourse.tile import TileContext
from concourse.bass_utils import run_bass_kernel_spmd

F32 = mybir.dt.float32
BF16 = mybir.dt.bfloat16
AF = mybir.ActivationFunctionType
OP = mybir.AluOpType

C = 256          # channels
S = 4096         # spatial (64*64)
SQ = 1024        # spatial quarter handled per core
H = 4            # heads
D = 64           # head dim
G = 32           # groups
EPS = 1e-6
NT = 32          # t tiles of 128 over S
VW = D + 2       # v' width per head (v | ones | pad), 4B-aligned blocks


def build_nc():
    nc = bass.Bass("TRN2", target_bir_lowering=False, debug=False, num_devices=8)

    def din(name, shape, dt=F32):
        return nc.dram_tensor(name, shape, dt, kind="ExternalInput").ap()

    x_d = din("x", [C, S], BF16)    # full batch slice, for stats + k/v
    y_d = din("y", [C, S], BF16)    # full batch slice, for stats
    xq_d = din("xq", [C, SQ])       # spatial quarter of x (residual, f32)
    yq_d = din("yq", [C, SQ], BF16)  # spatial quarter of y (queries)
    wqT_d = din("wqT", [C, C], BF16)   # wq.T / 8 (q scale folded), bf16
    wkT_d = din("wkT", [C, C], BF16)
    wvT_d = din("wvT", [C, C], BF16)
    woT_d = din("woT", [C, C], BF16)
    # packed per-channel vectors: cols = (bq8, bo2, g1, b1, g2, b2)
    vecs_d = din("vecs", [C, 6])
    pool_d = din("poolm", [C, G])   # (c//8==g)/8
    exp_d = din("expandm", [G, C])  # (c//8==g)
    out_d = nc.dram_tensor("out", [C, SQ], F32, kind="ExternalOutput").ap()
    rcd = [nc.dram_tensor(f"rcd{i}", [1, SQ], F32).ap() for i in range(2)]

    with TileContext(nc) as tc:
        with (
            tc.tile_pool(name="pers", bufs=1) as pers,
            tc.tile_pool(name="sb1", bufs=1) as sb1,
            tc.tile_pool(name="sb2", bufs=2) as sb2,
            tc.tile_pool(name="expp", bufs=2) as expp,
            tc.tile_pool(name="ps", bufs=1, space="PSUM") as ps,
        ):
            # ---- persistent tiles -------------------------------------
            xq = [pers.tile([128, SQ], F32, tag=f"xq{m}", name=f"xq{m}")
                  for m in range(2)]
            yn = [pers.tile([128, SQ], BF16, tag=f"yn{m}", name=f"yn{m}")
                  for m in range(2)]
            xn = [pers.tile([128, S], BF16, tag=f"xn{m}", name=f"xn{m}")
                  for m in range(2)]
            k_sb = [[pers.tile([128, 1024], BF16, tag=f"ksb{m}_{j}",
                               name=f"ksb{m}_{j}") for j in range(4)]
                    for m in range(2)]
            q_sb = [pers.tile([128, SQ], BF16, tag=f"qsb{m}", name=f"qsb{m}")
                    for m in range(2)]
            v_sb = [pers.tile([128, 8 * H * VW], BF16, tag=f"vsb{j}",
                              name=f"vsb{j}") for j in range(4)]
            out_ds = [pers.tile([128, SQ], BF16, tag=f"ods{m}", name=f"ods{m}")
                      for m in range(2)]
            wq_b = [pers.tile([128, C], BF16, tag=f"wqb{m}", name=f"wqb{m}")
                    for m in range(2)]
            wk_b = [pers.tile([128, C], BF16, tag=f"wkb{m}", name=f"wkb{m}")
                    for m in range(2)]
            wv_b = [pers.tile([128, C], BF16, tag=f"wvb{m}", name=f"wvb{m}")
                    for m in range(2)]
            wo_b = [pers.tile([128, C], BF16, tag=f"wob{m}", name=f"wob{m}")
                    for m in range(2)]
            vecs = [pers.tile([128, 6], F32, tag=f"vecs{m}", name=f"vecs{m}")
                    for m in range(2)]
            # gb[name][m] -> [128, 1] column views of the packed vecs tile
            _vc = {"bq8": 0, "bo2": 1, "g1": 2, "b1": 3, "g2": 4, "b2": 5}
            gb = {nm: [vecs[m][:, i:i + 1] for m in range(2)]
                  for nm, i in _vc.items()}
            den32 = pers.tile([32, 64], F32, tag="den32", name="den32")
            rc32 = pers.tile([32, 64], F32, tag="rc32", name="rc32")

            # ones column (64) + pad (65) of each v' head block
            for j in range(4):
                vview = v_sb[j][:].rearrange("p (t h e) -> p t h e", t=8, h=H)
                nc.gpsimd.memset(vview[:, :, :, D:D + 2], 1.0)


            # ---- stage 1: inputs + group-norm stats --------------------
            with tc.tile_pool(name="big", bufs=1) as big:
                xf = [big.tile([128, S], BF16, tag=f"xf{m}", name=f"xf{m}")
                      for m in range(2)]
                yf = [big.tile([128, S], BF16, tag=f"yf{m}", name=f"yf{m}")
                      for m in range(2)]
                yqf = [big.tile([128, SQ], BF16, tag=f"yqf{m}",
                                name=f"yqf{m}") for m in range(2)]
                s6x = [sb1.tile([128, 48], F32, tag=f"s6x{m}", name=f"s6x{m}")
                       for m in range(2)]
                s6y = [sb1.tile([128, 48], F32, tag=f"s6y{m}", name=f"s6y{m}")
                       for m in range(2)]

                # x first (k/v gate the pipeline), chunked DMA + stats
                for m in range(2):
                    cs = slice(m * 128, (m + 1) * 128)
                    for ch in range(4):
                        fs = slice(ch * 1024, (ch + 1) * 1024)
                        nc.sync.dma_start(out=xf[m][:, fs], in_=x_d[cs, fs])
                        for h2 in range(2):
                            c8 = 2 * ch + h2
                            nc.vector.bn_stats(
                                s6x[m][:, c8 * 6:(c8 + 1) * 6],
                                xf[m][:, c8 * 512:(c8 + 1) * 512])
                for m in range(2):
                    cs = slice(m * 128, (m + 1) * 128)
                    for ch in range(4):
                        fs = slice(ch * 1024, (ch + 1) * 1024)
                        nc.sync.dma_start(out=yf[m][:, fs], in_=y_d[cs, fs])
                        for h2 in range(2):
                            c8 = 2 * ch + h2
                            nc.vector.bn_stats(
                                s6y[m][:, c8 * 6:(c8 + 1) * 6],
                                yf[m][:, c8 * 512:(c8 + 1) * 512])

                for m in range(2):
                    nc.sync.dma_start(out=vecs[m][:],
                                      in_=vecs_d[m * 128:(m + 1) * 128, :])
                pool_sb = [sb1.tile([128, G], F32, tag=f"pl{m}", name=f"pl{m}")
                           for m in range(2)]
                expand_sb = sb1.tile([G, C], F32, tag="ex", name="ex")
                for m in range(2):
                    nc.sync.dma_start(out=pool_sb[m][:],
                                      in_=pool_d[m * 128:(m + 1) * 128, :])
                nc.sync.dma_start(out=expand_sb[:], in_=exp_d[:])
                for m in range(2):
                    cs = slice(m * 128, (m + 1) * 128)
                    nc.sync.dma_start(out=yqf[m][:], in_=yq_d[cs, :])
                    nc.sync.dma_start(out=xq[m][:], in_=xq_d[cs, :])
                    nc.sync.dma_start(out=wq_b[m][:], in_=wqT_d[cs, :])
                    nc.sync.dma_start(out=wk_b[m][:], in_=wkT_d[cs, :])
                    nc.sync.dma_start(out=wv_b[m][:], in_=wvT_d[cs, :])
                    nc.sync.dma_start(out=wo_b[m][:], in_=woT_d[cs, :])

                def group_affine(s6, gamma, beta, tagp, ve):
                    """per-channel A, B [128,1] x2 tiles from bn_stats
                    chunks; ve picks the engine for the small elementwise
                    ops (DVE for x, GpSimd for y so the chains overlap)"""
                    stats_c = []
                    for m in range(2):
                        mv = sb1.tile([128, 2], F32, tag=f"mv{tagp}{m}",
                                      name=f"mv{tagp}{m}")
                        nc.vector.bn_aggr(mv[:], s6[m][:])
                        st = sb1.tile([128, 2], F32, tag=f"st{tagp}{m}",
                                      name=f"st{tagp}{m}")
                        ve.tensor_copy(st[:, 0:1], mv[:, 0:1])
                        msq = sb1.tile([128, 1], F32, tag=f"msq{tagp}{m}",
                                       name=f"msq{tagp}{m}")
                        ve.tensor_mul(msq[:], mv[:, 0:1], mv[:, 0:1])
                        ve.tensor_add(st[:, 1:2], mv[:, 1:2], msq[:])
                        stats_c.append(st)
                    gp = ps.tile([G, 2], F32, tag="psD", padded_shape=[128, 1024], name=f"gp{tagp}")
                    for m in range(2):
                        nc.tensor.matmul(gp[:], lhsT=pool_sb[m][:],
                                         rhs=stats_c[m][:],
                                         start=(m == 0), stop=(m == 1))
                    gs = sb1.tile([G, 2], F32, tag=f"gs{tagp}",
                                  name=f"gs{tagp}")
                    nc.vector.tensor_copy(gs[:], gp[:])
                    musq = sb1.tile([G, 1], F32, tag=f"gmusq{tagp}",
                                    name=f"gmusq{tagp}")
                    ve.tensor_mul(musq[:], gs[:, 0:1], gs[:, 0:1])
                    veps = sb1.tile([G, 1], F32, tag=f"veps{tagp}",
                                    name=f"veps{tagp}")
                    ve.tensor_sub(veps[:], gs[:, 1:2], musq[:])
                    ve.tensor_scalar_add(veps[:], veps[:], EPS)
                    sq = sb1.tile([G, 1], F32, tag=f"gsq{tagp}",
                                  name=f"gsq{tagp}")
                    nc.scalar.sqrt(sq[:], veps[:])
                    r0 = sb1.tile([G, 1], F32, tag=f"gr0{tagp}",
                                  name=f"gr0{tagp}")
                    nc.vector.reciprocal(r0[:], sq[:])
                    # Newton polish: r1 = r0*(1.5 - 0.5*veps*r0^2)
                    y2 = sb1.tile([G, 1], F32, tag=f"gy2{tagp}",
                                  name=f"gy2{tagp}")
                    ve.tensor_mul(y2[:], r0[:], r0[:])
                    ve.tensor_mul(y2[:], veps[:], y2[:])
                    ve.tensor_scalar(y2[:], y2[:], -0.5, 1.5,
                                     OP.mult, OP.add)
                    gs2 = sb1.tile([G, 2], F32, tag=f"gs2{tagp}",
                                   name=f"gs2{tagp}")
                    ve.tensor_mul(gs2[:, 0:1], r0[:], y2[:])
                    ve.tensor_copy(gs2[:, 1:2], gs[:, 0:1])
                    A, B = [], []
                    for m in range(2):
                        pc = ps.tile([128, 2], F32, tag="psD", padded_shape=[128, 1024],
                                     name=f"pc{tagp}{m}")
                        nc.tensor.matmul(
                            pc[:], lhsT=expand_sb[:, m * 128:(m + 1) * 128],
                            rhs=gs2[:], start=True, stop=True)
                        a = sb1.tile([128, 1], F32, tag=f"A{tagp}{m}",
                                     name=f"A{tagp}{m}")
                        nc.vector.tensor_mul(a[:], pc[:, 0:1], gamma[m])
                        bmid = sb1.tile([128, 1], F32, tag=f"Bm{tagp}{m}",
                                        name=f"Bm{tagp}{m}")
                        nc.vector.tensor_mul(bmid[:], pc[:, 1:2], a[:])
                        b_ = sb1.tile([128, 1], F32, tag=f"B{tagp}{m}",
                                      name=f"B{tagp}{m}")
                        ve.tensor_sub(b_[:], beta[m], bmid[:])
                        A.append(a)
                        B.append(b_)
                    return A, B

                Ax, Bx = group_affine(s6x, gb["g1"], gb["b1"], "x",
                                      nc.vector)
                # normalize x: tile 0 on DVE, tile 1 on ACT (both idle here)
                nc.vector.tensor_scalar(xn[0][:], xf[0][:], Ax[0][:, 0:1],
                                        Bx[0][:, 0:1], OP.mult, OP.add)
                nc.gpsimd.tensor_scalar(xn[1][:], xf[1][:], Ax[1][:, 0:1],
                                        Bx[1][:, 0:1], OP.mult, OP.add)
                Ay, By = group_affine(s6y, gb["g2"], gb["b2"], "y",
                                      nc.gpsimd)
                nc.vector.tensor_scalar(yn[0][:], yqf[0][:], Ay[0][:, 0:1],
                                        By[0][:, 0:1], OP.mult, OP.add)
                nc.gpsimd.tensor_scalar(yn[1][:], yqf[1][:], Ay[1][:, 0:1],
                                        By[1][:, 0:1], OP.mult, OP.add)

            # ---- stage 2: qkv projections ------------------------------
            for m in range(2):
                pq = ps.tile([128, SQ], F32, tag="psA", name=f"pq{m}")
                for n in range(0, SQ, 512):
                    for kk in range(2):
                        nc.tensor.matmul(
                            pq[:, n:n + 512],
                            lhsT=wq_b[kk][:, m * 128:(m + 1) * 128],
                            rhs=yn[kk][:, n:n + 512],
                            start=(kk == 0), stop=(kk == 1))
                nc.vector.tensor_scalar_add(q_sb[m][:], pq[:],
                                            gb["bq8"][m])
            for m in range(2):
                for n in range(0, S, 512):
                    pk = ps.tile([128, 512], F32,
                                 tag="psB" if (n // 512) % 2 == 0 else "psC",
                                 padded_shape=[128, 1024], name=f"pk{m}_{n}")
                    for kk in range(2):
                        nc.tensor.matmul(
                            pk[:],
                            lhsT=wk_b[kk][:, m * 128:(m + 1) * 128],
                            rhs=xn[kk][:, n:n + 512],
                            start=(kk == 0), stop=(kk == 1))
                    # bk is a no-op through softmax (per-s shift): plain
                    # copies, alternated ScalarE / VectorE
                    kdst = k_sb[m][n // 1024][:, n % 1024:n % 1024 + 512]
                    if (n // 512) % 2 == 0:
                        nc.scalar.copy(kdst, pk[:])
                    else:
                        nc.vector.tensor_copy(kdst, pk[:])
            # PE warm-up: a continuous burst keeps the HAM clock-gate at
            # full rate entering the attention loop (it tends to stick cold
            # after idle gaps otherwise).
            wu = ps.tile([128, 512], F32, tag="psC", padded_shape=[128, 1024],
                         name="wu")
            for i in range(16):
                nc.tensor.matmul(wu[:], lhsT=wq_b[0][:, 0:128],
                                 rhs=xn[0][:, 0:512], start=(i == 0),
                                 stop=(i == 15))

            # ---- stage 3: attention ------------------------------------
            po = []
            for p in range(2):
                sc = [ps.tile([128, SQ], F32, tag=["psA", "psB"][hh],
                              name=f"sc{p}_{hh}") for hh in range(2)]
                acc = [ps.tile([VW, SQ], F32, tag=["psC", "psD"][hh],
                               padded_shape=[128, 1024],
                               name=f"acc{p}_{hh}") for hh in range(2)]
                def emit_scores(hh, t):
                    tsl = slice((t % 8) * 128, (t % 8 + 1) * 128)
                    lo = hh * 64
                    for n in range(0, SQ, 512):
                        nc.tensor.matmul(
                            sc[hh][:, n:n + 512],
                            lhsT=k_sb[p][t // 8][lo:lo + 64, tsl],
                            rhs=q_sb[p][lo:lo + 64, n:n + 512],
                            start=True, stop=True)

                # scores run one tile ahead of exp/attnv so the in-order PE
                # always has ready work while an exp is in flight
                for hh in range(2):
                    emit_scores(hh, 0)
                if p == 0:
                    # v projection emitted after the scores prologue: the
                    # first exps outprioritize it; its matmuls fill PE slack
                    # during the early exps
                    for t in range(NT):
                        pv = ps.tile([128, C], F32,
                                     tag="psC" if t % 2 == 0 else "psD",
                                     padded_shape=[128, 1024], name=f"pv{t}")
                        tsl = slice(t * 128, (t + 1) * 128)
                        for kk in range(2):
                            nc.tensor.matmul(pv[:], lhsT=xn[kk][:, tsl],
                                             rhs=wv_b[kk][:],
                                             start=(kk == 0), stop=(kk == 1))
                        pvv = pv[:].rearrange("p (h e) -> p h e", h=H)
                        dst = v_sb[t // 8][:, (t % 8) * H * VW:
                                           (t % 8 + 1) * H * VW]
                        dvv = dst.rearrange("p (h e) -> p h e",
                                            h=H)[:, :, 0:D]
                        nc.vector.tensor_copy(dvv, pvv)
                for t in range(NT):
                    for hh in range(2):
                        h = 2 * p + hh
                        e = expp.tile([128, SQ], BF16, tag=f"exp{hh}",
                                      name=f"e{p}_{hh}")
                        nc.scalar.activation(e[:], sc[hh][:], AF.Exp)
                        if t + 1 < NT:
                            emit_scores(hh, t + 1)
                        voff = (t % 8) * H * VW + h * VW
                        for n in range(0, SQ, 512):
                            nc.tensor.matmul(
                                acc[hh][:, n:n + 512],
                                lhsT=v_sb[t // 8][:, voff:voff + VW],
                                rhs=e[:, n:n + 512],
                                start=(t == 0), stop=(t == NT - 1))
                if p == 1:
                    # out_ds[0] has been final since pair 0: start the wo
                    # accumulation on it while this pair drains
                    for mo in range(2):
                        po_t = ps.tile([128, SQ], F32,
                                       tag="psA" if mo == 0 else "psB",
                                       name=f"po{mo}")
                        po.append(po_t)
                        for n in range(0, SQ, 512):
                            nc.tensor.matmul(
                                po_t[:, n:n + 512],
                                lhsT=wo_b[0][:, mo * 128:(mo + 1) * 128],
                                rhs=out_ds[0][:, n:n + 512],
                                start=True, stop=False)
                # drain the pair: normalize by the ones-column denominator
                # inter-pair PE filler: keeps the HAM clock-gate warm while
                # ACT drains the last exps and the accumulators are copied
                if p == 0:
                    wu2 = ps.tile([128, 512], F32, tag="psB",
                                  padded_shape=[128, 1024], name="wu2")
                    for i in range(10):
                        nc.tensor.matmul(wu2[:], lhsT=wq_b[0][:, 0:128],
                                         rhs=xn[0][:, 0:512], start=(i == 0),
                                         stop=(i == 9))
                # free both accumulator slots first (pair p+1's attnv
                # waits on them through the in-order PE queue)
                asbs = []
                for hh in range(2):
                    asb = sb2.tile([VW, SQ], F32, tag="asb", name="asb")
                    if hh == 0:
                        nc.vector.tensor_copy(asb[:], acc[hh][:])
                    else:
                        nc.scalar.copy(asb[:], acc[hh][:])
                    asbs.append(asb)
                # fused reciprocal for both heads: DMA spreads each
                # denominator row across 32 partitions (cross-partition
                # reshape), one DVE reciprocal covers both heads
                for hh in range(2):
                    nc.sync.dma_start(
                        out=den32[:, hh * 32:(hh + 1) * 32],
                        in_=asbs[hh][D:D + 1, :])
                nc.vector.reciprocal(rc32[:], den32[:])
                for hh in range(2):
                    # broadcast recip row to 64 partitions via a DRAM
                    # round-trip (stride-0 DRAM reads are legal for DMA)
                    nc.sync.dma_start(out=rcd[hh][:],
                                      in_=rc32[:, hh * 32:(hh + 1) * 32])
                    rbc = sb2.tile([64, SQ], F32, tag="rbc", name="rbc")
                    nc.sync.dma_start(out=rbc[:],
                                      in_=rcd[hh][0:1, :].broadcast_to(
                                          [64, SQ]))
                    if hh == 0:
                        nc.vector.tensor_mul(out_ds[p][0:64, :],
                                             asbs[hh][0:D, :], rbc[:])
                    else:
                        hsh = sb2.tile([64, SQ], BF16, tag="hsh", name="hsh")
                        nc.vector.tensor_mul(hsh[:], asbs[hh][0:D, :], rbc[:])
                        nc.sync.dma_start(out=out_ds[p][64:128, :],
                                           in_=hsh[:])

            # ---- stage 4: output projection + residual -----------------
            for mo in range(2):
                for n in range(0, SQ, 512):
                    nc.tensor.matmul(
                        po[mo][:, n:n + 512],
                        lhsT=wo_b[1][:, mo * 128:(mo + 1) * 128],
                        rhs=out_ds[1][:, n:n + 512],
                        start=False, stop=True)
                osb = sb2.tile([128, SQ], F32, tag="osb", name="osb")
                # bo2 is pre-added into xq on the host: single fused add
                nc.vector.tensor_add(osb[:], po[mo][:], xq[mo][:])
                nc.sync.dma_start(out=out_d[mo * 128:(mo + 1) * 128, :],
                                  in_=osb[:])

    # Legalize sync waits for this walrus build: at most one wait per
    # instruction (two on EventSemaphore) - same passes Bacc.compile runs.
    _br.move_matmul_waits_to_ldweights(nc.m)
    _br.generate_event_semaphores(nc)
    return nc


# ---------------------------------------------------------------------------
# Host-side constants + input prep
# ---------------------------------------------------------------------------
def _consts():
    cidx = np.arange(C)
    pool = np.zeros((C, G), np.float32)
    pool[cidx, cidx // 8] = 1.0 / 8.0
    expand = np.zeros((G, C), np.float32)
    expand[cidx // 8, cidx] = 1.0
    return pool, expand


def make_in_maps(x, y, g1, b1, g2, b2, wq, bq, wk, bk, wv, bv, wo, bo):
    f = lambda a: np.ascontiguousarray(np.asarray(a, dtype=np.float32))
    bf = lambda a: np.ascontiguousarray(np.asarray(a).astype(ml_dtypes.bfloat16))
    x = f(x).reshape(2, C, S)
    y = f(y).reshape(2, C, S)
    xb16 = x.astype(ml_dtypes.bfloat16)
    yb16 = y.astype(ml_dtypes.bfloat16)
    pool, expand = _consts()
    col = lambda a: np.ascontiguousarray(f(a).reshape(C, 1))
    bo2 = f(bo) + f(wo) @ f(bv)   # softmax-average commutes the v bias
    vecs = np.stack([f(bq) / 8.0, bo2, f(g1), f(b1), f(g2), f(b2)],
                    axis=1).astype(np.float32)
    base = {
        "wqT": bf(f(wq).T / 8.0),
        "wkT": bf(f(wk).T),
        "wvT": bf(f(wv).T),
        "woT": bf(f(wo).T),
        "vecs": np.ascontiguousarray(vecs),
        "poolm": pool, "expandm": expand,
    }
    in_maps = []
    for core in range(8):
        b, sq = core // 4, core % 4
        m = dict(base)
        m["x"] = np.ascontiguousarray(xb16[b])
        m["y"] = np.ascontiguousarray(yb16[b])
        m["xq"] = np.ascontiguousarray(
            x[b][:, sq * SQ:(sq + 1) * SQ] + bo2[:, None].astype(np.float32))
        m["yq"] = np.ascontiguousarray(yb16[b][:, sq * SQ:(sq + 1) * SQ])
        in_maps.append(m)
    return in_maps


_NC_CACHE = None


def _get_nc():
    global _NC_CACHE
    if _NC_CACHE is None:
        _NC_CACHE = build_nc()
    return _NC_CACHE


def kernel(**inputs) -> np.ndarray:
    nc = _get_nc()
    in_maps = make_in_maps(**inputs)
    res = run_bass_kernel_spmd(nc, in_maps, core_ids=list(range(8)))
    out = np.empty((2, C, S), np.float32)
    for core in range(8):
        b, sq = core // 4, core % 4
        out[b][:, sq * SQ:(sq + 1) * SQ] = res.results[core]["out"]
    return out.reshape(2, C, 64, 64)



# revision 13
# speedup vs baseline: 1.0565x; 1.0565x over previous
"""MultiHeadAttnBlock TRN2 kernel.

Full inputs -> shard across 8 NeuronCores -> full output.

Sharding: core i handles (batch b = i//4, spatial quarter sq = i%4).
Each core computes group-norm stats for its batch, normalizes x/y with the
per-channel affine (A, B) derived from the group stats, computes K/V over
the full spatial dim and Q over its quarter, runs 4-head attention for its
1024 query positions against all 4096 keys, projects with wo, and adds the
residual.  The host slices inputs and concatenates the 8 [256, 1024]
outputs.

Layout:
 - q, k in [c, s] "conv layout" straight out of the 1x1-conv matmul.
 - scores computed transposed: scT[t, s] = k[d, t-tile].T @ q[d, s];
   the two heads of a pair live at partitions 0-63 / 64-127 and share the
   PE array via row tiling.
 - exp on ScalarE - the kernel bottleneck (16.8M exps/core).
 - attn@v: out.T[d', s] = v'[t, d'].T @ expT[t, s] accumulated over the 32
   t-tiles in PSUM, where v' = [v | ones]: column 64 accumulates the
   softmax denominator for free.
 - the denominator reciprocal uses the DVE 32x32-transpose trick to spread
   4096 values across 32 lanes; the broadcast back to 64 partitions is a
   K=1 matmul written into the (already drained) accumulator PSUM so no
   extra PSUM bank is needed and the next pair's tiles are never blocked.
"""

import numpy as np
import ml_dtypes

import concourse.bass as bass
import concourse.mybir as mybir
import bass_rust as _br
from concourse.tile import TileContext
from concourse.bass_utils import run_bass_kernel_spmd

F32 = mybir.dt.float32
BF16 = mybir.dt.bfloat16
AF = mybir.ActivationFunctionType
OP = mybir.AluOpType

C = 256          # channels
S = 4096         # spatial (64*64)
SQ = 1024        # spatial quarter handled per core
H = 4            # heads
D = 64           # head dim
G = 32           # groups
EPS = 1e-6
NT = 32          # t tiles of 128 over S
VW = 80          # v' width per head (v | ones | pad), 16B-aligned fp8 blocks
VM = D + 1       # v' matmul columns actually used (v | ones)
F8 = mybir.dt.float8e4
EXP_BIAS = -2.5  # softmax-invariant shift so exp() fits fp8e4 range (<240)


def build_nc():
    nc = bass.Bass("TRN2", target_bir_lowering=False, debug=False, num_devices=8)

    def din(name, shape, dt=F32):
        return nc.dram_tensor(name, shape, dt, kind="ExternalInput").ap()

    x_d = din("x", [C, S], BF16)    # full batch slice, for stats + k/v
    y_d = din("y", [C, S], BF16)    # full batch slice, for stats
    xq_d = din("xq", [C, SQ])       # spatial quarter of x (residual, f32)
    yq_d = din("yq", [C, SQ], BF16)  # spatial quarter of y (queries)
    wqT_d = din("wqT", [C, C], BF16)   # wq.T / 8 (q scale folded), bf16
    wkT_d = din("wkT", [C, C], BF16)
    wvT_d = din("wvT", [C, C], BF16)
    woT_d = din("woT", [C, C], BF16)
    # packed per-channel vectors: cols = (bq8, bo2, g1, b1, g2, b2)
    vecs_d = din("vecs", [C, 6])
    pool_d = din("poolm", [C, G])   # (c//8==g)/8
    exp_d = din("expandm", [G, C])  # (c//8==g)
    out_d = nc.dram_tensor("out", [C, SQ], F32, kind="ExternalOutput").ap()
    rcd = [nc.dram_tensor(f"rcd{i}", [1, SQ], F32).ap() for i in range(2)]

    with TileContext(nc) as tc:
        with (
            tc.tile_pool(name="pers", bufs=1) as pers,
            tc.tile_pool(name="sb1", bufs=1) as sb1,
            tc.tile_pool(name="sb2", bufs=2) as sb2,
            tc.tile_pool(name="expp", bufs=2) as expp,
            tc.tile_pool(name="ps", bufs=1, space="PSUM") as ps,
        ):
            # ---- persistent tiles -------------------------------------
            xq = [pers.tile([128, SQ], F32, tag=f"xq{m}", name=f"xq{m}")
                  for m in range(2)]
            yn = [pers.tile([128, SQ], BF16, tag=f"yn{m}", name=f"yn{m}")
                  for m in range(2)]
            xn = [pers.tile([128, S], BF16, tag=f"xn{m}", name=f"xn{m}")
                  for m in range(2)]
            k_sb = [[pers.tile([128, 1024], BF16, tag=f"ksb{m}_{j}",
                               name=f"ksb{m}_{j}") for j in range(4)]
                    for m in range(2)]
            q_sb = [pers.tile([128, SQ], BF16, tag=f"qsb{m}", name=f"qsb{m}")
                    for m in range(2)]
            v_sb = [pers.tile([128, 8 * H * VW], F8, tag=f"vsb{j}",
                              name=f"vsb{j}") for j in range(4)]
            out_ds = [pers.tile([128, SQ], BF16, tag=f"ods{m}", name=f"ods{m}")
                      for m in range(2)]
            wq_b = [pers.tile([128, C], BF16, tag=f"wqb{m}", name=f"wqb{m}")
                    for m in range(2)]
            wk_b = [pers.tile([128, C], BF16, tag=f"wkb{m}", name=f"wkb{m}")
                    for m in range(2)]
            wv_b = [pers.tile([128, C], BF16, tag=f"wvb{m}", name=f"wvb{m}")
                    for m in range(2)]
            wo_b = [pers.tile([128, C], BF16, tag=f"wob{m}", name=f"wob{m}")
                    for m in range(2)]
            vecs = [pers.tile([128, 6], F32, tag=f"vecs{m}", name=f"vecs{m}")
                    for m in range(2)]
            # gb[name][m] -> [128, 1] column views of the packed vecs tile
            _vc = {"bq8": 0, "bo2": 1, "g1": 2, "b1": 3, "g2": 4, "b2": 5}
            gb = {nm: [vecs[m][:, i:i + 1] for m in range(2)]
                  for nm, i in _vc.items()}
            den32 = pers.tile([32, 64], F32, tag="den32", name="den32")
            rc32 = pers.tile([32, 64], F32, tag="rc32", name="rc32")
            ebias = pers.tile([128, 1], F32, tag="ebias", name="ebias")
            nc.gpsimd.memset(ebias[:], EXP_BIAS)

            # ones column (64) of each v' head block (fp8 exact)
            for j in range(4):
                vview = v_sb[j][:].rearrange("p (t h e) -> p t h e", t=8, h=H)
                nc.gpsimd.memset(vview[:, :, :, D:D + 1], 1.0)


            # ---- stage 1: inputs + group-norm stats --------------------
            with tc.tile_pool(name="big", bufs=1) as big:
                xf = [big.tile([128, S], BF16, tag=f"xf{m}", name=f"xf{m}")
                      for m in range(2)]
                yf = [big.tile([128, S], BF16, tag=f"yf{m}", name=f"yf{m}")
                      for m in range(2)]
                yqf = [big.tile([128, SQ], BF16, tag=f"yqf{m}",
                                name=f"yqf{m}") for m in range(2)]
                s6x = [sb1.tile([128, 48], F32, tag=f"s6x{m}", name=f"s6x{m}")
                       for m in range(2)]
                s6y = [sb1.tile([128, 48], F32, tag=f"s6y{m}", name=f"s6y{m}")
                       for m in range(2)]

                # x first (k/v gate the pipeline), chunked DMA + stats
                for m in range(2):
                    cs = slice(m * 128, (m + 1) * 128)
                    for ch in range(4):
                        fs = slice(ch * 1024, (ch + 1) * 1024)
                        nc.sync.dma_start(out=xf[m][:, fs], in_=x_d[cs, fs])
                        for h2 in range(2):
                            c8 = 2 * ch + h2
                            nc.vector.bn_stats(
                                s6x[m][:, c8 * 6:(c8 + 1) * 6],
                                xf[m][:, c8 * 512:(c8 + 1) * 512])
                for m in range(2):
                    cs = slice(m * 128, (m + 1) * 128)
                    for ch in range(4):
                        fs = slice(ch * 1024, (ch + 1) * 1024)
                        nc.sync.dma_start(out=yf[m][:, fs], in_=y_d[cs, fs])
                        for h2 in range(2):
                            c8 = 2 * ch + h2
                            nc.vector.bn_stats(
                                s6y[m][:, c8 * 6:(c8 + 1) * 6],
                                yf[m][:, c8 * 512:(c8 + 1) * 512])

                for m in range(2):
                    nc.sync.dma_start(out=vecs[m][:],
                                      in_=vecs_d[m * 128:(m + 1) * 128, :])
                pool_sb = [sb1.tile([128, G], F32, tag=f"pl{m}", name=f"pl{m}")
                           for m in range(2)]
                expand_sb = sb1.tile([G, C], F32, tag="ex", name="ex")
                for m in range(2):
                    nc.sync.dma_start(out=pool_sb[m][:],
                                      in_=pool_d[m * 128:(m + 1) * 128, :])
                nc.sync.dma_start(out=expand_sb[:], in_=exp_d[:])
                for m in range(2):
                    cs = slice(m * 128, (m + 1) * 128)
                    nc.sync.dma_start(out=yqf[m][:], in_=yq_d[cs, :])
                    nc.sync.dma_start(out=xq[m][:], in_=xq_d[cs, :])
                    nc.sync.dma_start(out=wq_b[m][:], in_=wqT_d[cs, :])
                    nc.sync.dma_start(out=wk_b[m][:], in_=wkT_d[cs, :])
                    nc.sync.dma_start(out=wv_b[m][:], in_=wvT_d[cs, :])
                    nc.sync.dma_start(out=wo_b[m][:], in_=woT_d[cs, :])

                def group_affine(s6, gamma, beta, tagp, ve):
                    """per-channel A, B [128,1] x2 tiles from bn_stats
                    chunks; ve picks the engine for the small elementwise
                    ops (DVE for x, GpSimd for y so the chains overlap)"""
                    stats_c = []
                    for m in range(2):
                        mv = sb1.tile([128, 2], F32, tag=f"mv{tagp}{m}",
                                      name=f"mv{tagp}{m}")
                        nc.vector.bn_aggr(mv[:], s6[m][:])
                        st = sb1.tile([128, 2], F32, tag=f"st{tagp}{m}",
                                      name=f"st{tagp}{m}")
                        ve.tensor_copy(st[:, 0:1], mv[:, 0:1])
                        msq = sb1.tile([128, 1], F32, tag=f"msq{tagp}{m}",
                                       name=f"msq{tagp}{m}")
                        ve.tensor_mul(msq[:], mv[:, 0:1], mv[:, 0:1])
                        ve.tensor_add(st[:, 1:2], mv[:, 1:2], msq[:])
                        stats_c.append(st)
                    gp = ps.tile([G, 2], F32, tag="psD", padded_shape=[128, 1024], name=f"gp{tagp}")
                    for m in range(2):
                        nc.tensor.matmul(gp[:], lhsT=pool_sb[m][:],
                                         rhs=stats_c[m][:],
                                         start=(m == 0), stop=(m == 1))
                    gs = sb1.tile([G, 2], F32, tag=f"gs{tagp}",
                                  name=f"gs{tagp}")
                    nc.vector.tensor_copy(gs[:], gp[:])
                    musq = sb1.tile([G, 1], F32, tag=f"gmusq{tagp}",
                                    name=f"gmusq{tagp}")
                    ve.tensor_mul(musq[:], gs[:, 0:1], gs[:, 0:1])
                    veps = sb1.tile([G, 1], F32, tag=f"veps{tagp}",
                                    name=f"veps{tagp}")
                    ve.tensor_sub(veps[:], gs[:, 1:2], musq[:])
                    ve.tensor_scalar_add(veps[:], veps[:], EPS)
                    sq = sb1.tile([G, 1], F32, tag=f"gsq{tagp}",
                                  name=f"gsq{tagp}")
                    nc.scalar.sqrt(sq[:], veps[:])
                    r0 = sb1.tile([G, 1], F32, tag=f"gr0{tagp}",
                                  name=f"gr0{tagp}")
                    nc.vector.reciprocal(r0[:], sq[:])
                    # Newton polish: r1 = r0*(1.5 - 0.5*veps*r0^2)
                    y2 = sb1.tile([G, 1], F32, tag=f"gy2{tagp}",
                                  name=f"gy2{tagp}")
                    ve.tensor_mul(y2[:], r0[:], r0[:])
                    ve.tensor_mul(y2[:], veps[:], y2[:])
                    ve.tensor_scalar(y2[:], y2[:], -0.5, 1.5,
                                     OP.mult, OP.add)
                    gs2 = sb1.tile([G, 2], F32, tag=f"gs2{tagp}",
                                   name=f"gs2{tagp}")
                    ve.tensor_mul(gs2[:, 0:1], r0[:], y2[:])
                    ve.tensor_copy(gs2[:, 1:2], gs[:, 0:1])
                    A, B = [], []
                    for m in range(2):
                        pc = ps.tile([128, 2], F32, tag="psD", padded_shape=[128, 1024],
                                     name=f"pc{tagp}{m}")
                        nc.tensor.matmul(
                            pc[:], lhsT=expand_sb[:, m * 128:(m + 1) * 128],
                            rhs=gs2[:], start=True, stop=True)
                        a = sb1.tile([128, 1], F32, tag=f"A{tagp}{m}",
                                     name=f"A{tagp}{m}")
                        nc.vector.tensor_mul(a[:], pc[:, 0:1], gamma[m])
                        bmid = sb1.tile([128, 1], F32, tag=f"Bm{tagp}{m}",
                                        name=f"Bm{tagp}{m}")
                        nc.vector.tensor_mul(bmid[:], pc[:, 1:2], a[:])
                        b_ = sb1.tile([128, 1], F32, tag=f"B{tagp}{m}",
                                      name=f"B{tagp}{m}")
                        ve.tensor_sub(b_[:], beta[m], bmid[:])
                        A.append(a)
                        B.append(b_)
                    return A, B

                Ax, Bx = group_affine(s6x, gb["g1"], gb["b1"], "x",
                                      nc.vector)
                # normalize x: tile 0 on DVE, tile 1 on ACT (both idle here)
                nc.vector.tensor_scalar(xn[0][:], xf[0][:], Ax[0][:, 0:1],
                                        Bx[0][:, 0:1], OP.mult, OP.add)
                nc.gpsimd.tensor_scalar(xn[1][:], xf[1][:], Ax[1][:, 0:1],
                                        Bx[1][:, 0:1], OP.mult, OP.add)
                Ay, By = group_affine(s6y, gb["g2"], gb["b2"], "y",
                                      nc.gpsimd)
                nc.vector.tensor_scalar(yn[0][:], yqf[0][:], Ay[0][:, 0:1],
                                        By[0][:, 0:1], OP.mult, OP.add)
                nc.gpsimd.tensor_scalar(yn[1][:], yqf[1][:], Ay[1][:, 0:1],
                                        By[1][:, 0:1], OP.mult, OP.add)

            # ---- stage 2: qkv projections ------------------------------
            for m in range(2):
                pq = ps.tile([128, SQ], F32, tag="psA", name=f"pq{m}")
                for n in range(0, SQ, 512):
                    for kk in range(2):
                        nc.tensor.matmul(
                            pq[:, n:n + 512],
                            lhsT=wq_b[kk][:, m * 128:(m + 1) * 128],
                            rhs=yn[kk][:, n:n + 512],
                            start=(kk == 0), stop=(kk == 1))
                nc.vector.tensor_scalar_add(q_sb[m][:], pq[:],
                                            gb["bq8"][m])
            for m in range(2):
                for n in range(0, S, 512):
                    pk = ps.tile([128, 512], F32,
                                 tag="psB" if (n // 512) % 2 == 0 else "psC",
                                 padded_shape=[128, 1024], name=f"pk{m}_{n}")
                    for kk in range(2):
                        nc.tensor.matmul(
                            pk[:],
                            lhsT=wk_b[kk][:, m * 128:(m + 1) * 128],
                            rhs=xn[kk][:, n:n + 512],
                            start=(kk == 0), stop=(kk == 1))
                    # bk is a no-op through softmax (per-s shift): plain
                    # copies, alternated ScalarE / VectorE
                    kdst = k_sb[m][n // 1024][:, n % 1024:n % 1024 + 512]
                    if (n // 512) % 2 == 0:
                        nc.scalar.copy(kdst, pk[:])
                    else:
                        nc.vector.tensor_copy(kdst, pk[:])
            # PE warm-up: a continuous burst keeps the HAM clock-gate at
            # full rate entering the attention loop (it tends to stick cold
            # after idle gaps otherwise).
            wu = ps.tile([128, 512], F32, tag="psC", padded_shape=[128, 1024],
                         name="wu")
            for i in range(16):
                nc.tensor.matmul(wu[:], lhsT=wq_b[0][:, 0:128],
                                 rhs=xn[0][:, 0:512], start=(i == 0),
                                 stop=(i == 15))

            # ---- stage 3: attention ------------------------------------
            po = []
            for p in range(2):
                sc = [ps.tile([128, SQ], F32, tag=["psA", "psB"][hh],
                              name=f"sc{p}_{hh}") for hh in range(2)]
                acc = [ps.tile([VM, SQ], F32, tag=["psC", "psD"][hh],
                               padded_shape=[128, 1024],
                               name=f"acc{p}_{hh}") for hh in range(2)]
                def emit_scores(hh, t):
                    tsl = slice((t % 8) * 128, (t % 8 + 1) * 128)
                    lo = hh * 64
                    for n in range(0, SQ, 512):
                        nc.tensor.matmul(
                            sc[hh][:, n:n + 512],
                            lhsT=k_sb[p][t // 8][lo:lo + 64, tsl],
                            rhs=q_sb[p][lo:lo + 64, n:n + 512],
                            start=True, stop=True)

                # scores run one tile ahead of exp/attnv so the in-order PE
                # always has ready work while an exp is in flight
                for hh in range(2):
                    emit_scores(hh, 0)
                if p == 0:
                    # v projection emitted after the scores prologue: the
                    # first exps outprioritize it; its matmuls fill PE slack
                    # during the early exps
                    for t in range(NT):
                        pv = ps.tile([128, C], F32,
                                     tag="psC" if t % 2 == 0 else "psD",
                                     padded_shape=[128, 1024], name=f"pv{t}")
                        tsl = slice(t * 128, (t + 1) * 128)
                        for kk in range(2):
                            nc.tensor.matmul(pv[:], lhsT=xn[kk][:, tsl],
                                             rhs=wv_b[kk][:],
                                             start=(kk == 0), stop=(kk == 1))
                        pvv = pv[:].rearrange("p (h e) -> p h e", h=H)
                        dst = v_sb[t // 8][:, (t % 8) * H * VW:
                                           (t % 8 + 1) * H * VW]
                        dvv = dst.rearrange("p (h e) -> p h e",
                                            h=H)[:, :, 0:D]
                        nc.vector.tensor_copy(dvv, pvv)
                epair = [None, None]
                for t in range(NT):
                    for hh in range(2):
                        h = 2 * p + hh
                        if t % 2 == 0:
                            epair[hh] = expp.tile([128, 2 * SQ], F8,
                                                  tag=f"exp{hh}",
                                                  name=f"e{p}_{hh}_{t//2}")
                        e = epair[hh]
                        # fp8 e: bias shifts scores so exp() stays in e4m3
                        # range; softmax-invariant (cancels in the average)
                        nc.scalar.activation(
                            e[:, (t % 2) * SQ:(t % 2 + 1) * SQ], sc[hh][:],
                            AF.Exp, bias=ebias[:, 0:1])
                        if t + 1 < NT:
                            emit_scores(hh, t + 1)
                        if t % 2 == 1:
                            # t-pair complete: DoubleRow fp8 attn@v over 256
                            # virtual contraction rows (2 t-tiles per matmul)
                            u = t // 2
                            tl = (2 * u) % 8
                            vv = v_sb[(2 * u) // 8][:].rearrange(
                                "q (t2 h2 e2) -> q t2 h2 e2", t2=8, h2=H)
                            v3 = vv[:, tl:tl + 2, h, 0:VM]
                            e3 = e[:].rearrange("q (j n) -> q j n", j=2)
                            for n in range(0, SQ, 512):
                                nc.tensor.matmul(
                                    acc[hh][:, n:n + 512],
                                    lhsT=v3, rhs=e3[:, :, n:n + 512],
                                    start=(u == 0), stop=(u == NT // 2 - 1),
                                    perf_mode=mybir.MatmulPerfMode.DoubleRow)
                if p == 1:
                    # out_ds[0] has been final since pair 0: start the wo
                    # accumulation on it while this pair drains
                    for mo in range(2):
                        po_t = ps.tile([128, SQ], F32,
                                       tag="psA" if mo == 0 else "psB",
                                       name=f"po{mo}")
                        po.append(po_t)
                        for n in range(0, SQ, 512):
                            nc.tensor.matmul(
                                po_t[:, n:n + 512],
                                lhsT=wo_b[0][:, mo * 128:(mo + 1) * 128],
                                rhs=out_ds[0][:, n:n + 512],
                                start=True, stop=False)
                # drain the pair: normalize by the ones-column denominator
                # inter-pair PE filler: keeps the HAM clock-gate warm while
                # ACT drains the last exps and the accumulators are copied
                if p == 0:
                    wu2 = ps.tile([128, 512], F32, tag="psB",
                                  padded_shape=[128, 1024], name="wu2")
                    for i in range(10):
                        nc.tensor.matmul(wu2[:], lhsT=wq_b[0][:, 0:128],
                                         rhs=xn[0][:, 0:512], start=(i == 0),
                                         stop=(i == 9))
                # free both accumulator slots first (pair p+1's attnv
                # waits on them through the in-order PE queue)
                asbs = []
                for hh in range(2):
                    asb = sb2.tile([VM, SQ], F32, tag="asb", name="asb")
                    nc.vector.tensor_copy(asb[:], acc[hh][:])
                    asbs.append(asb)
                # fused reciprocal for both heads: DMA spreads each
                # denominator row across 32 partitions (cross-partition
                # reshape), one DVE reciprocal covers both heads
                for hh in range(2):
                    nc.sync.dma_start(
                        out=den32[:, hh * 32:(hh + 1) * 32],
                        in_=asbs[hh][D:D + 1, :])
                nc.vector.reciprocal(rc32[:], den32[:])
                for hh in range(2):
                    # broadcast recip row to 64 partitions via a DRAM
                    # round-trip (stride-0 DRAM reads are legal for DMA)
                    nc.sync.dma_start(out=rcd[hh][:],
                                      in_=rc32[:, hh * 32:(hh + 1) * 32])
                    rbc = sb2.tile([64, SQ], F32, tag="rbc", name="rbc")
                    nc.sync.dma_start(out=rbc[:],
                                      in_=rcd[hh][0:1, :].broadcast_to(
                                          [64, SQ]))
                    if hh == 0:
                        nc.vector.tensor_mul(out_ds[p][0:64, :],
                                             asbs[hh][0:D, :], rbc[:])
                    else:
                        hsh = sb2.tile([64, SQ], BF16, tag="hsh", name="hsh")
                        nc.vector.tensor_mul(hsh[:], asbs[hh][0:D, :], rbc[:])
                        nc.sync.dma_start(out=out_ds[p][64:128, :],
                                           in_=hsh[:])

            # ---- stage 4: output projection + residual -----------------
            for mo in range(2):
                for n in range(0, SQ, 512):
                    nc.tensor.matmul(
                        po[mo][:, n:n + 512],
                        lhsT=wo_b[1][:, mo * 128:(mo + 1) * 128],
                        rhs=out_ds[1][:, n:n + 512],
                        start=False, stop=True)
                osb = sb2.tile([128, SQ], F32, tag="osb", name="osb")
                # bo2 is pre-added into xq on the host: single fused add
                nc.vector.tensor_add(osb[:], po[mo][:], xq[mo][:])
                nc.sync.dma_start(out=out_d[mo * 128:(mo + 1) * 128, :],
                                  in_=osb[:])

    # Legalize sync waits for this walrus build: at most one wait per
    # instruction (two on EventSemaphore) - same passes Bacc.compile runs.
    _br.move_matmul_waits_to_ldweights(nc.m)
    _br.generate_event_semaphores(nc)
    return nc


# ---------------------------------------------------------------------------
# Host-side constants + input prep
# ---------------------------------------------------------------------------
def _consts():
    cidx = np.arange(C)
    pool = np.zeros((C, G), np.float32)
    pool[cidx, cidx // 8] = 1.0 / 8.0
    expand = np.zeros((G, C), np.float32)
    expand[cidx // 8, cidx] = 1.0
    return pool, expand


def make_in_maps(x, y, g1, b1, g2, b2, wq, bq, wk, bk, wv, bv, wo, bo):
    f = lambda a: np.ascontiguousarray(np.asarray(a, dtype=np.float32))
    bf = lambda a: np.ascontiguousarray(np.asarray(a).astype(ml_dtypes.bfloat16))
    x = f(x).reshape(2, C, S)
    y = f(y).reshape(2, C, S)
    xb16 = x.astype(ml_dtypes.bfloat16)
    yb16 = y.astype(ml_dtypes.bfloat16)
    pool, expand = _consts()
    col = lambda a: np.ascontiguousarray(f(a).reshape(C, 1))
    bo2 = f(bo) + f(wo) @ f(bv)   # softmax-average commutes the v bias
    vecs = np.stack([f(bq) / 8.0, bo2, f(g1), f(b1), f(g2), f(b2)],
                    axis=1).astype(np.float32)
    base = {
        "wqT": bf(f(wq).T / 8.0),
        "wkT": bf(f(wk).T),
        "wvT": bf(f(wv).T),
        "woT": bf(f(wo).T),
        "vecs": np.ascontiguousarray(vecs),
        "poolm": pool, "expandm": expand,
    }
    in_maps = []
    for core in range(8):
        b, sq = core // 4, core % 4
        m = dict(base)
        m["x"] = np.ascontiguousarray(xb16[b])
        m["y"] = np.ascontiguousarray(yb16[b])
        m["xq"] = np.ascontiguousarray(
            x[b][:, sq * SQ:(sq + 1) * SQ] + bo2[:, None].astype(np.float32))
        m["yq"] = np.ascontiguousarray(yb16[b][:, sq * SQ:(sq + 1) * SQ])
        in_maps.append(m)
    return in_maps


_NC_CACHE = None


def _get_nc():
    global _NC_CACHE
    if _NC_CACHE is None:
        _NC_CACHE = build_nc()
    return _NC_CACHE


def kernel(**inputs) -> np.ndarray:
    nc = _get_nc()
    in_maps = make_in_maps(**inputs)
    res = run_bass_kernel_spmd(nc, in_maps, core_ids=list(range(8)))
    out = np.empty((2, C, S), np.float32)
    for core in range(8):
        b, sq = core // 4, core % 4
        out[b][:, sq * SQ:(sq + 1) * SQ] = res.results[core]["out"]
    return out.reshape(2, C, 64, 64)

